# revision 31
# baseline (speedup 1.0000x reference)
"""Trainium2 Bass kernel for the sliding-window additive-attention layer.

Reference (L=4096, D=H=512, P=16):
    score[l, d] = Wv . tanh(wx[l] + u[l+d]),  d in [-16..16]\\{0}
    g[l] = softmax_d(score[l, :]) . x_window

Algorithm: tanh is replaced by a 2-harmonic separable sine expansion.
With X = w0*wx, Y = w0*u (w0 = 2pi/9 folded into the projection weights
on the host), theta = X + pi/4, and the identities
    sin(X+Y)  =  sin(X+pi/4)sin(Y+pi/4) - sin(X-pi/4)sin(Y-pi/4)
    sin(3t)   =  sin(t) (3 - 4 sin^2(t))          (DVE triple-angle)
    sin^2(X+pi/4) + sin^2(X-pi/4) = 1             (shared square)
the fit  tanh(wx+u) ~= sum_m c_m sin(m(X+Y)) + f(wx)  (harmonics m=1,3;
pure-f(wx) terms cancel in the softmax over the window) turns the score
tensor into a BANDED MATMUL between per-position trig factor tensors:
    score[l, l'] = sum_h A_k[h, l] * Bs_k[h, l']    (4 products k)
eliminating the [L, 32, H] tanh entirely.  Per 128-row l-block the
[128, 160] score band is exp'ed (ACT), band-masked (GPSIMD), row-summed
(DVE), transposed (PE), matmul'ed against the halo rows of x (PE), and
normalized during the psum->sbuf copy (ACT Copy with scale=1/Z).

Engine split per core: PE fp8-DoubleRow projections + banded scores +
output matmuls; ACT 4 Sin evals per h-chunk + exp + normalize; DVE
triple-angle harmonics + Wv-scaled copies + row sums; GPSIMD band
masking.  Sequence-parallel over 8 cores with 16-row halos;
sharding/assembly is host-side in kernel().
"""

import numpy as np
import ml_dtypes

import concourse.bass as bass
import concourse.mybir as mybir
import concourse.tile as tile
from concourse import bacc, bass_utils

BF16 = mybir.dt.bfloat16
FP8 = mybir.dt.float8e4
F32 = mybir.dt.float32
AF = mybir.ActivationFunctionType
ALU = mybir.AluOpType

L, D, H, P = 4096, 512, 512, 16
M = 8                  # cores
LLOC = L // M          # 512 rows per core
HALO = LLOC + 2 * P    # 544
NDC = D // 128         # 4 d-chunks
NHC = H // 128         # 4 h-chunks
NLC = LLOC // 128      # 4 l-blocks
BAND = 160             # l' window per l-block (128 + 2P)

W0 = 2.0 * np.pi / 9.0
PH = float(np.pi / 4)
FS = 64.0              # fp8 weight pre-scale (keeps w0*W out of subnormals)
# product coefficients for (A1+,B1+), (A1-,B1-), (A3+,B3+), (A3-,B3-)
COEF = [1.05419824, -1.05366673, -0.10666493, 0.10645345]
R2 = COEF[2] / COEF[0]   # harmonic-3 factors derive from the scaled
R3 = COEF[3] / COEF[1]   # harmonic-1 tiles via these coefficient ratios


def build_nc() -> bass.Bass:
    nc = bacc.Bacc("TRN2", target_bir_lowering=False, debug=False)

    xT_d = nc.dram_tensor("xT", [128, 2, 2, HALO], FP8, kind="ExternalInput")
    xh_d = nc.dram_tensor("xh", [128, NLC + 1, D], BF16, kind="ExternalInput")
    wwT_d = nc.dram_tensor("wwT", [128, 2, 2, H], FP8, kind="ExternalInput")
    wuT_d = nc.dram_tensor("wuT", [128, 2, 2, H], FP8, kind="ExternalInput")
    wvc_d = nc.dram_tensor("wvc", [128, NHC, 4], F32, kind="ExternalInput")
    mask_d = nc.dram_tensor("mask", [128, BAND], BF16, kind="ExternalInput")
    eye_d = nc.dram_tensor("eye", [128, 128], BF16, kind="ExternalInput")
    out_d = nc.dram_tensor("out", [128, NLC, D], BF16, kind="ExternalOutput")

    with tile.TileContext(nc) as tc:
        with (
            tc.tile_pool(name="persist", bufs=1) as pp,
            tc.tile_pool(name="btmp", bufs=2) as bt_pool,
            tc.tile_pool(name="wr", bufs=2) as wr_pool,
            tc.tile_pool(name="tail", bufs=2) as tl_pool,
            tc.tile_pool(name="pA", bufs=1, space="PSUM") as psA,
            tc.tile_pool(name="pBIG", bufs=4, space="PSUM") as psBIG,
            tc.tile_pool(name="pTP", bufs=2, space="PSUM") as psTP,
        ):
            # ---- persistent SBUF ----
            xT_sb = pp.tile([128, 2, 2, HALO], FP8, tag="xT")
            xh_sb = pp.tile([128, NLC + 1, D], BF16, tag="xh")
            wwT_sb = pp.tile([128, 2, 2, H], FP8, tag="wwT")
            wuT_sb = pp.tile([128, 2, 2, H], FP8, tag="wuT")
            wvc_sb = pp.tile([128, NHC, 4], F32, tag="wvc")
            mask_sb = pp.tile([128, BAND], BF16, tag="mask")
            eye_sb = pp.tile([128, 128], BF16, tag="eye")
            php = pp.tile([128, 1], F32, tag="php")
            phm = pp.tile([128, 1], F32, tag="phm")
            A_sb = pp.tile([128, NHC, 4, LLOC], BF16, tag="A")
            Bs_sb = pp.tile([128, NHC, 4, HALO], BF16, tag="Bs")
            gout_sb = pp.tile([128, NLC, D], BF16, tag="gout")
            z_sb = pp.tile([128, NLC], F32, tag="z")
            rz_sb = pp.tile([128, NLC], F32, tag="rz")

            nc.vector.memset(php[:, :], PH)
            nc.vector.memset(phm[:, :], -PH)

            # ---- input DMAs over three queues; u-path inputs first ----
            nc.scalar.dma_start(wuT_sb[:, 0, :, :], wuT_d[:, 0, :, :])
            nc.scalar.dma_start(wuT_sb[:, 1, :, :], wuT_d[:, 1, :, :])
            nc.sync.dma_start(xT_sb[:, 0, :, :], xT_d[:, 0, :, :])
            nc.sync.dma_start(xT_sb[:, 1, :, :], xT_d[:, 1, :, :])
            nc.sync.dma_start(wwT_sb[:, :, :, :], wwT_d[:, :, :, :])
            nc.sync.dma_start(wvc_sb[:, :, :], wvc_d[:, :, :])
            nc.sync.dma_start(eye_sb[:, :], eye_d[:, :])
            nc.sync.dma_start(mask_sb[:, :], mask_d[:, :])
            # xh is only needed by the tail output matmuls -> slow queue is fine
            nc.gpsimd.dma_start(xh_sb[:, :, :], xh_d[:, :, :])


            # all four l-blocks' score psums live in one 4-buffer pool of
            # [128, 512] f32 tiles; the same buffers are recycled for the
            # output-matmul psums once the exps have read the scores
            sc_tiles = [psBIG.tile([128, D], F32, tag="big", name=f"sc{i}")
                        for i in range(4)]

            def sc_chunk(lb, hc, k):
                ls = slice(128 * lb, 128 * lb + 128)
                bs = slice(128 * lb, 128 * lb + BAND)
                nc.tensor.matmul(
                    sc_tiles[lb][:, 0:BAND], A_sb[:, hc, k, ls],
                    Bs_sb[:, hc, k, bs],
                    start=(hc == 0 and k == 0), stop=(hc == NHC - 1 and k == 3),
                )

            acm_tiles = [None] * NLC

            def emit_softmax(lb):
                ac = tl_pool.tile([128, BAND], BF16, tag="ac", bufs=4,
                                  name=f"ac{lb}")
                nc.scalar.activation(ac[:, :], sc_tiles[lb][:, 0:BAND], AF.Exp)
                acm = tl_pool.tile([128, BAND], BF16, tag="acm", bufs=4,
                                   name=f"acm{lb}")
                nc.gpsimd.tensor_mul(acm[:, :], ac[:, :], mask_sb[:, :])
                acm_tiles[lb] = acm
                nc.vector.tensor_reduce(
                    z_sb[:, lb:lb + 1], acm[:, :],
                    axis=mybir.AxisListType.X, op=ALU.add,
                )
                nc.vector.reciprocal(rz_sb[:, lb:lb + 1], z_sb[:, lb:lb + 1])

            def emit_gather(lb):
                acm = acm_tiles[lb]
                # both band transposes land in disjoint regions of one psum
                # tile; one copy moves them to SBUF together
                at_ps = psTP.tile([128, 256], BF16, tag="at")
                nc.tensor.transpose(at_ps[:, 0:128], acm[:, 0:128], eye_sb[:, :])
                nc.tensor.transpose(at_ps[0:32, 128:256], acm[:, 128:BAND], eye_sb[:, :])
                at = tl_pool.tile([128, 256], BF16, tag="ats")
                nc.vector.tensor_copy(at[:, :], at_ps[:, :])
                g_ps = psBIG.tile([128, D], F32, tag="big", name=f"g{lb}")
                nc.tensor.matmul(g_ps[:, :], at[:, 0:128], xh_sb[:, lb, :],
                                 start=True, stop=False)
                nc.tensor.matmul(g_ps[:, :], at[0:32, 128:256], xh_sb[0:32, lb + 1, :],
                                 start=False, stop=True)
                # normalize by 1/Z during the psum->sbuf copy; split in
                # halves so the out-DMA overlaps the second half
                nc.scalar.activation(gout_sb[:, lb, 0:256], g_ps[:, 0:256],
                                     AF.Copy, scale=rz_sb[:, lb:lb + 1])
                nc.sync.dma_start(out_d[:, lb, 0:256], gout_sb[:, lb, 0:256])
                nc.scalar.activation(gout_sb[:, lb, 256:512], g_ps[:, 256:512],
                                     AF.Copy, scale=rz_sb[:, lb:lb + 1])
                nc.sync.dma_start(out_d[:, lb, 256:512], gout_sb[:, lb, 256:512])

            # ---- per h-chunk: projections + trig factors + score chunks ----
            for hc in range(NHC):
                hs = slice(128 * hc, 128 * hc + 128)
                # u first: its psum buffer is freed by the b1 sins quickly
                u_ps = psA.tile([128, 512], F32, tag="u")
                for s in range(2):
                    nc.tensor.matmul(
                        u_ps[:, :], wuT_sb[:, s, :, hs], xT_sb[:, s, :, 0:512],
                        start=(s == 0), stop=(s == 1),
                        perf_mode=mybir.MatmulPerfMode.DoubleRow,
                    )
                # the 32-col u tail borrows unused psum columns of sc_tiles[3]
                # (block 3's scores only accumulate after the hc loop, by which
                # point every tail has been consumed by its sins)
                ut_ps = sc_tiles[3][:, 480:512]
                for s in range(2):
                    nc.tensor.matmul(
                        ut_ps, wuT_sb[:, s, :, hs],
                        xT_sb[:, s, :, 512:HALO],
                        start=(s == 0), stop=(s == 1),
                        perf_mode=mybir.MatmulPerfMode.DoubleRow,
                    )
                wx_ps = psA.tile([128, LLOC], F32, tag="wx")
                for s in range(2):
                    nc.tensor.matmul(
                        wx_ps[:, :], wwT_sb[:, s, :, hs],
                        xT_sb[:, s, :, P:P + LLOC],
                        start=(s == 0), stop=(s == 1),
                        perf_mode=mybir.MatmulPerfMode.DoubleRow,
                    )

                # harmonic-1 factors (w0 folded into weights on host)
                b1p = bt_pool.tile([128, HALO], BF16, tag="b1p")
                b1m = bt_pool.tile([128, HALO], BF16, tag="b1m")
                nc.scalar.activation(b1p[:, 0:512], u_ps[:, :], AF.Sin, bias=php[:, :], scale=1.0 / FS)
                nc.scalar.activation(b1p[:, 512:HALO], ut_ps, AF.Sin, bias=php[:, :], scale=1.0 / FS)
                nc.scalar.activation(b1m[:, 0:512], u_ps[:, :], AF.Sin, bias=phm[:, :], scale=1.0 / FS)
                nc.scalar.activation(b1m[:, 512:HALO], ut_ps, AF.Sin, bias=phm[:, :], scale=1.0 / FS)
                nc.scalar.activation(A_sb[:, hc, 0, :], wx_ps[:, :], AF.Sin, bias=php[:, :], scale=1.0 / FS)
                nc.scalar.activation(A_sb[:, hc, 1, :], wx_ps[:, :], AF.Sin, bias=phm[:, :], scale=1.0 / FS)

                # harmonic 3 on DVE: sin(3t) = sin(t)(3-4sin^2 t); the two
                # phases share one square, and on the b-side the harmonic-3
                # moving factors derive from the already-Wv-scaled harmonic-1
                # tiles via the coefficient ratios R2/R3 (signs cancel with
                # the negated a-side k=3 factor)
                tb = wr_pool.tile([128, HALO], BF16, tag="tb")
                nc.vector.tensor_mul(tb[:, :], b1p[:, :], b1p[:, :])
                ta = wr_pool.tile([128, LLOC], BF16, tag="ta")
                nc.vector.tensor_mul(ta[:, :], A_sb[:, hc, 0, :], A_sb[:, hc, 0, :])
                # k=0,1 b-factors first so their score chunks start early
                nc.vector.tensor_scalar_mul(
                    Bs_sb[:, hc, 0, :], b1p[:, :], wvc_sb[:, hc, 0:1])
                nc.vector.tensor_scalar_mul(
                    Bs_sb[:, hc, 1, :], b1m[:, :], wvc_sb[:, hc, 1:2])
                v2 = wr_pool.tile([128, HALO], BF16, tag="v2")
                nc.vector.tensor_scalar(v2[:, :], tb[:, :], -4.0 * R2, 3.0 * R2,
                                        ALU.mult, ALU.add)
                v3 = wr_pool.tile([128, HALO], BF16, tag="v3")
                nc.vector.tensor_scalar(v3[:, :], tb[:, :], -4.0 * R3, 1.0 * R3,
                                        ALU.mult, ALU.add)
                nc.vector.tensor_mul(Bs_sb[:, hc, 2, :], Bs_sb[:, hc, 0, :], v2[:, :])
                nc.vector.tensor_mul(Bs_sb[:, hc, 3, :], Bs_sb[:, hc, 1, :], v3[:, :])
                vap = wr_pool.tile([128, LLOC], BF16, tag="vap")
                nc.vector.tensor_scalar(vap[:, :], ta[:, :], -4.0, 3.0, ALU.mult, ALU.add)
                nc.vector.tensor_mul(A_sb[:, hc, 2, :], A_sb[:, hc, 0, :], vap[:, :])
                nc.vector.scalar_tensor_tensor(
                    A_sb[:, hc, 3, :], vap[:, :], 2.0, A_sb[:, hc, 1, :],
                    op0=ALU.subtract, op1=ALU.mult)

                # stream score contributions for the PREVIOUS hc (its Bs/A
                # factors are complete by now) for l-blocks 0-1; blocks 2-3
                # run after the loop so the loop stays DVE-bound, not PE-bound
                if hc > 0:
                    for k in range(4):
                        for lb in range(3):
                            sc_chunk(lb, hc - 1, k)

            # ---- remaining chunks, then the two tail stages ----
            # l-block 3's first three hc's chunks are ready now; they fill
            # PE's wait on the last hc's DVE chain
            for hc in range(NHC - 1):
                for k in range(4):
                    sc_chunk(3, hc, k)
            # lb-major for the gated group: block 0 completes ~1us earlier,
            # starting the softmax/gather chain sooner
            for lb in range(NLC):
                for k in range(4):
                    sc_chunk(lb, NHC - 1, k)
                emit_softmax(lb)
            for lb in range(NLC):
                emit_gather(lb)

    nc.compile()
    return nc


def make_in_maps(x, Ww, Wu, Wv):
    bf = ml_dtypes.bfloat16
    x = np.asarray(x, np.float32)
    x_pad = np.zeros((L + 2 * P, D), np.float32)
    x_pad[P:P + L] = x

    f8 = ml_dtypes.float8_e4m3
    # [d, h] -> [p, s, i, h] with d = 256 s + 128 i + p, fp8 with FS pre-scale
    wwT = np.ascontiguousarray((FS * W0 * np.asarray(Ww, np.float32)).T).astype(f8)
    wwT = wwT.reshape(2, 2, 128, H).transpose(2, 0, 1, 3)
    wuT = np.ascontiguousarray((FS * W0 * np.asarray(Wu, np.float32)).T).astype(f8)
    wuT = wuT.reshape(2, 2, 128, H).transpose(2, 0, 1, 3)

    wv = np.asarray(Wv, np.float32)[0]
    wvc = np.zeros((128, NHC, 4), np.float32)
    for hc in range(NHC):
        for k in range(4):
            wvc[:, hc, k] = COEF[k] * wv[128 * hc:128 * hc + 128]

    jj = np.arange(BAND)[None, :]
    ll = np.arange(128)[:, None]
    dd = jj - ll
    mask = (((dd >= 0) & (dd <= 2 * P)) & (dd != P)).astype(bf)

    eye = np.eye(128, dtype=bf)

    in_maps = []
    for m in range(M):
        xh = x_pad[LLOC * m: LLOC * m + HALO].astype(bf)
        xh_a = np.zeros((128, NLC + 1, D), bf)
        xh_a[:, :NLC] = xh[:512].reshape(NLC, 128, D).transpose(1, 0, 2)
        xh_a[0:32, NLC] = xh[512:HALO]
        xT = np.ascontiguousarray(x_pad[LLOC * m: LLOC * m + HALO].T).astype(f8)
        xT_a = xT.reshape(2, 2, 128, HALO).transpose(2, 0, 1, 3)
        in_maps.append({
            "xT": np.ascontiguousarray(xT_a),
            "xh": np.ascontiguousarray(xh_a),
            "wwT": np.ascontiguousarray(wwT),
            "wuT": np.ascontiguousarray(wuT),
            "wvc": wvc,
            "mask": np.ascontiguousarray(mask),
            "eye": eye,
        })
    return in_maps


def assemble_out(results):
    shards = []
    for m in range(M):
        o = np.asarray(results[m]["out"]).reshape(128, NLC, D)
        shards.append(o.transpose(1, 0, 2).reshape(LLOC, D))
    return np.concatenate(shards, 0).astype(np.float32)


def kernel(x, Ww, Wu, Wv):
    nc = build_nc()
    in_maps = make_in_maps(x, Ww, Wu, Wv)
    res = bass_utils.run_bass_kernel_spmd(nc, in_maps, core_ids=list(range(M)))
    return assemble_out(res.results)


# revision 32
# speedup vs baseline: 1.0183x; 1.0183x over previous
"""Trainium2 Bass kernel for the sliding-window additive-attention layer.

Reference (L=4096, D=H=512, P=16):
    score[l, d] = Wv . tanh(wx[l] + u[l+d]),  d in [-16..16]\\{0}
    g[l] = softmax_d(score[l, :]) . x_window

Algorithm: tanh is replaced by a 2-harmonic separable sine expansion.
With X = w0*wx, Y = w0*u (w0 = 2pi/9 folded into the projection weights
on the host), theta = X + pi/4, and the identities
    sin(X+Y)  =  sin(X+pi/4)sin(Y+pi/4) - sin(X-pi/4)sin(Y-pi/4)
    sin(3t)   =  sin(t) (3 - 4 sin^2(t))          (DVE triple-angle)
    sin^2(X+pi/4) + sin^2(X-pi/4) = 1             (shared square)
the fit  tanh(wx+u) ~= sum_m c_m sin(m(X+Y)) + f(wx)  (harmonics m=1,3;
pure-f(wx) terms cancel in the softmax over the window) turns the score
tensor into a BANDED MATMUL between per-position trig factor tensors:
    score[l, l'] = sum_h A_k[h, l] * Bs_k[h, l']    (4 products k)
eliminating the [L, 32, H] tanh entirely.  Per 128-row l-block the
[128, 160] score band is exp'ed (ACT), band-masked (GPSIMD), row-summed
(DVE), transposed (PE), matmul'ed against the halo rows of x (PE), and
normalized during the psum->sbuf copy (ACT Copy with scale=1/Z).

Engine split per core: PE fp8-DoubleRow projections + banded scores +
output matmuls; ACT 4 Sin evals per h-chunk + exp + normalize; DVE
triple-angle harmonics + Wv-scaled copies + row sums; GPSIMD band
masking.  Sequence-parallel over 8 cores with 16-row halos;
sharding/assembly is host-side in kernel().
"""

import numpy as np
import ml_dtypes

import concourse.bass as bass
import concourse.mybir as mybir
import concourse.tile as tile
from concourse import bacc, bass_utils

BF16 = mybir.dt.bfloat16
FP8 = mybir.dt.float8e4
F32 = mybir.dt.float32
AF = mybir.ActivationFunctionType
ALU = mybir.AluOpType

L, D, H, P = 4096, 512, 512, 16
M = 8                  # cores
LLOC = L // M          # 512 rows per core
HALO = LLOC + 2 * P    # 544
NDC = D // 128         # 4 d-chunks
NHC = H // 128         # 4 h-chunks
NLC = LLOC // 128      # 4 l-blocks
BAND = 160             # l' window per l-block (128 + 2P)

W0 = 2.0 * np.pi / 9.0
PH = float(np.pi / 4)
FS = 64.0              # fp8 weight pre-scale (keeps w0*W out of subnormals)
# product coefficients for (A1+,B1+), (A1-,B1-), (A3+,B3+), (A3-,B3-)
COEF = [1.05419824, -1.05366673, -0.10666493, 0.10645345]
R2 = COEF[2] / COEF[0]   # harmonic-3 factors derive from the scaled
R3 = COEF[3] / COEF[1]   # harmonic-1 tiles via these coefficient ratios


def build_nc() -> bass.Bass:
    nc = bacc.Bacc("TRN2", target_bir_lowering=False, debug=False)

    xT_d = nc.dram_tensor("xT", [128, 2, 2, HALO], FP8, kind="ExternalInput")
    xh_d = nc.dram_tensor("xh", [128, NLC + 1, D], BF16, kind="ExternalInput")
    wwT_d = nc.dram_tensor("wwT", [128, 2, 2, H], FP8, kind="ExternalInput")
    wuT_d = nc.dram_tensor("wuT", [128, 2, 2, H], FP8, kind="ExternalInput")
    wvc_d = nc.dram_tensor("wvc", [128, NHC, 4], F32, kind="ExternalInput")
    mask_d = nc.dram_tensor("mask", [128, BAND], BF16, kind="ExternalInput")
    eye_d = nc.dram_tensor("eye", [128, 128], BF16, kind="ExternalInput")
    out_d = nc.dram_tensor("out", [128, NLC, D], BF16, kind="ExternalOutput")

    with tile.TileContext(nc) as tc:
        with (
            tc.tile_pool(name="persist", bufs=1) as pp,
            tc.tile_pool(name="btmp", bufs=2) as bt_pool,
            tc.tile_pool(name="wr", bufs=2) as wr_pool,
            tc.tile_pool(name="tail", bufs=2) as tl_pool,
            tc.tile_pool(name="pA", bufs=1, space="PSUM") as psA,
            tc.tile_pool(name="pBIG", bufs=4, space="PSUM") as psBIG,
            tc.tile_pool(name="pTP", bufs=2, space="PSUM") as psTP,
        ):
            # ---- persistent SBUF ----
            xT_sb = pp.tile([128, 2, 2, HALO], FP8, tag="xT")
            xh_sb = pp.tile([128, NLC + 1, D], BF16, tag="xh")
            wwT_sb = pp.tile([128, 2, 2, H], FP8, tag="wwT")
            wuT_sb = pp.tile([128, 2, 2, H], FP8, tag="wuT")
            wvc_sb = pp.tile([128, NHC, 4], F32, tag="wvc")
            mask_sb = pp.tile([128, BAND], BF16, tag="mask")
            eye_sb = pp.tile([128, 128], BF16, tag="eye")
            php = pp.tile([128, 1], F32, tag="php")
            phm = pp.tile([128, 1], F32, tag="phm")
            A_sb = pp.tile([128, NHC, 4, LLOC], BF16, tag="A")
            Bs_sb = pp.tile([128, NHC, 4, HALO], BF16, tag="Bs")
            gout_sb = pp.tile([128, NLC, D], BF16, tag="gout")
            z_sb = pp.tile([128, NLC], F32, tag="z")
            rz_sb = pp.tile([128, NLC], F32, tag="rz")

            nc.vector.memset(php[:, :], PH)
            nc.vector.memset(phm[:, :], -PH)

            # ---- input DMAs over three queues; u-path inputs first ----
            nc.scalar.dma_start(wuT_sb[:, 0, :, :], wuT_d[:, 0, :, :])
            nc.scalar.dma_start(wuT_sb[:, 1, :, :], wuT_d[:, 1, :, :])
            nc.sync.dma_start(xT_sb[:, 0, :, :], xT_d[:, 0, :, :])
            nc.sync.dma_start(xT_sb[:, 1, :, :], xT_d[:, 1, :, :])
            nc.sync.dma_start(wwT_sb[:, :, :, :], wwT_d[:, :, :, :])
            nc.sync.dma_start(wvc_sb[:, :, :], wvc_d[:, :, :])
            nc.sync.dma_start(eye_sb[:, :], eye_d[:, :])
            nc.sync.dma_start(mask_sb[:, :], mask_d[:, :])
            # xh is only needed by the tail output matmuls -> slow queue is fine
            nc.gpsimd.dma_start(xh_sb[:, :, :], xh_d[:, :, :])


            # all four l-blocks' score psums live in one 4-buffer pool of
            # [128, 512] f32 tiles; the same buffers are recycled for the
            # output-matmul psums once the exps have read the scores
            sc_tiles = [psBIG.tile([128, D], F32, tag="big", name=f"sc{i}")
                        for i in range(4)]

            def sc_chunk(lb, hc, k):
                ls = slice(128 * lb, 128 * lb + 128)
                bs = slice(128 * lb, 128 * lb + BAND)
                nc.tensor.matmul(
                    sc_tiles[lb][:, 0:BAND], A_sb[:, hc, k, ls],
                    Bs_sb[:, hc, k, bs],
                    start=(hc == 0 and k == 0), stop=(hc == NHC - 1 and k == 3),
                )

            acm_tiles = [None] * NLC

            def emit_softmax(lb):
                ac = tl_pool.tile([128, BAND], BF16, tag="ac", bufs=4,
                                  name=f"ac{lb}")
                nc.scalar.activation(ac[:, :], sc_tiles[lb][:, 0:BAND], AF.Exp)
                acm = tl_pool.tile([128, BAND], BF16, tag="acm", bufs=4,
                                   name=f"acm{lb}")
                nc.gpsimd.tensor_mul(acm[:, :], ac[:, :], mask_sb[:, :])
                acm_tiles[lb] = acm
                nc.vector.tensor_reduce(
                    z_sb[:, lb:lb + 1], acm[:, :],
                    axis=mybir.AxisListType.X, op=ALU.add,
                )
                nc.vector.reciprocal(rz_sb[:, lb:lb + 1], z_sb[:, lb:lb + 1])

            def emit_gather(lb):
                acm = acm_tiles[lb]
                # both band transposes land in disjoint regions of one psum
                # tile; one copy moves them to SBUF together
                at_ps = psTP.tile([128, 256], BF16, tag="at")
                nc.tensor.transpose(at_ps[:, 0:128], acm[:, 0:128], eye_sb[:, :])
                nc.tensor.transpose(at_ps[0:32, 128:256], acm[:, 128:BAND], eye_sb[:, :])
                at = tl_pool.tile([128, 256], BF16, tag="ats")
                nc.vector.tensor_copy(at[:, :], at_ps[:, :])
                g_ps = psBIG.tile([128, D], F32, tag="big", name=f"g{lb}")
                nc.tensor.matmul(g_ps[:, :], at[:, 0:128], xh_sb[:, lb, :],
                                 start=True, stop=False)
                nc.tensor.matmul(g_ps[:, :], at[0:32, 128:256], xh_sb[0:32, lb + 1, :],
                                 start=False, stop=True)
                # normalize by 1/Z during the psum->sbuf copy
                nc.scalar.activation(gout_sb[:, lb, :], g_ps[:, :], AF.Copy,
                                     scale=rz_sb[:, lb:lb + 1])
                nc.sync.dma_start(out_d[:, lb, :], gout_sb[:, lb, :])

            # ---- per h-chunk: projections + trig factors + score chunks ----
            for hc in range(NHC):
                hs = slice(128 * hc, 128 * hc + 128)
                # u first: its psum buffer is freed by the b1 sins quickly
                u_ps = psA.tile([128, 512], F32, tag="u")
                for s in range(2):
                    nc.tensor.matmul(
                        u_ps[:, :], wuT_sb[:, s, :, hs], xT_sb[:, s, :, 0:512],
                        start=(s == 0), stop=(s == 1),
                        perf_mode=mybir.MatmulPerfMode.DoubleRow,
                    )
                # the 32-col u tail borrows unused psum columns of sc_tiles[3]
                # (block 3's scores only accumulate after the hc loop, by which
                # point every tail has been consumed by its sins)
                ut_ps = sc_tiles[3][:, 480:512]
                for s in range(2):
                    nc.tensor.matmul(
                        ut_ps, wuT_sb[:, s, :, hs],
                        xT_sb[:, s, :, 512:HALO],
                        start=(s == 0), stop=(s == 1),
                        perf_mode=mybir.MatmulPerfMode.DoubleRow,
                    )
                wx_ps = psA.tile([128, LLOC], F32, tag="wx")
                for s in range(2):
                    nc.tensor.matmul(
                        wx_ps[:, :], wwT_sb[:, s, :, hs],
                        xT_sb[:, s, :, P:P + LLOC],
                        start=(s == 0), stop=(s == 1),
                        perf_mode=mybir.MatmulPerfMode.DoubleRow,
                    )

                # harmonic-1 factors (w0 folded into weights on host)
                b1p = bt_pool.tile([128, HALO], BF16, tag="b1p")
                b1m = bt_pool.tile([128, HALO], BF16, tag="b1m")
                nc.scalar.activation(b1p[:, 0:512], u_ps[:, :], AF.Sin, bias=php[:, :], scale=1.0 / FS)
                nc.scalar.activation(b1p[:, 512:HALO], ut_ps, AF.Sin, bias=php[:, :], scale=1.0 / FS)
                nc.scalar.activation(b1m[:, 0:512], u_ps[:, :], AF.Sin, bias=phm[:, :], scale=1.0 / FS)
                nc.scalar.activation(b1m[:, 512:HALO], ut_ps, AF.Sin, bias=phm[:, :], scale=1.0 / FS)
                nc.scalar.activation(A_sb[:, hc, 0, :], wx_ps[:, :], AF.Sin, bias=php[:, :], scale=1.0 / FS)
                nc.scalar.activation(A_sb[:, hc, 1, :], wx_ps[:, :], AF.Sin, bias=phm[:, :], scale=1.0 / FS)

                # harmonic 3 on DVE: sin(3t) = sin(t)(3-4sin^2 t); the two
                # phases share one square, and on the b-side the harmonic-3
                # moving factors derive from the already-Wv-scaled harmonic-1
                # tiles via the coefficient ratios R2/R3 (signs cancel with
                # the negated a-side k=3 factor)
                tb = wr_pool.tile([128, HALO], BF16, tag="tb")
                nc.vector.tensor_mul(tb[:, :], b1p[:, :], b1p[:, :])
                ta = wr_pool.tile([128, LLOC], BF16, tag="ta")
                nc.vector.tensor_mul(ta[:, :], A_sb[:, hc, 0, :], A_sb[:, hc, 0, :])
                # k=0,1 b-factors first so their score chunks start early
                nc.vector.tensor_scalar_mul(
                    Bs_sb[:, hc, 0, :], b1p[:, :], wvc_sb[:, hc, 0:1])
                nc.vector.tensor_scalar_mul(
                    Bs_sb[:, hc, 1, :], b1m[:, :], wvc_sb[:, hc, 1:2])
                v2 = wr_pool.tile([128, HALO], BF16, tag="v2")
                nc.vector.tensor_scalar(v2[:, :], tb[:, :], -4.0 * R2, 3.0 * R2,
                                        ALU.mult, ALU.add)
                v3 = wr_pool.tile([128, HALO], BF16, tag="v3")
                nc.vector.tensor_scalar(v3[:, :], tb[:, :], -4.0 * R3, 1.0 * R3,
                                        ALU.mult, ALU.add)
                nc.vector.tensor_mul(Bs_sb[:, hc, 2, :], Bs_sb[:, hc, 0, :], v2[:, :])
                nc.vector.tensor_mul(Bs_sb[:, hc, 3, :], Bs_sb[:, hc, 1, :], v3[:, :])
                vap = wr_pool.tile([128, LLOC], BF16, tag="vap")
                nc.vector.tensor_scalar(vap[:, :], ta[:, :], -4.0, 3.0, ALU.mult, ALU.add)
                nc.vector.tensor_mul(A_sb[:, hc, 2, :], A_sb[:, hc, 0, :], vap[:, :])
                nc.vector.scalar_tensor_tensor(
                    A_sb[:, hc, 3, :], vap[:, :], 2.0, A_sb[:, hc, 1, :],
                    op0=ALU.subtract, op1=ALU.mult)

                # stream score contributions for the PREVIOUS hc (its Bs/A
                # factors are complete by now) for l-blocks 0-1; blocks 2-3
                # run after the loop so the loop stays DVE-bound, not PE-bound
                if hc > 0:
                    for k in range(4):
                        for lb in range(3):
                            sc_chunk(lb, hc - 1, k)

            # ---- remaining chunks, then the two tail stages ----
            # l-block 3's first three hc's chunks are ready now; they fill
            # PE's wait on the last hc's DVE chain
            for hc in range(NHC - 1):
                for k in range(4):
                    sc_chunk(3, hc, k)
            for k in range(4):
                for lb in range(NLC):
                    sc_chunk(lb, NHC - 1, k)
            for lb in range(NLC):
                emit_softmax(lb)
            for lb in range(NLC):
                emit_gather(lb)

    nc.compile()
    return nc


def make_in_maps(x, Ww, Wu, Wv):
    bf = ml_dtypes.bfloat16
    x = np.asarray(x, np.float32)
    x_pad = np.zeros((L + 2 * P, D), np.float32)
    x_pad[P:P + L] = x

    f8 = ml_dtypes.float8_e4m3
    # [d, h] -> [p, s, i, h] with d = 256 s + 128 i + p, fp8 with FS pre-scale
    wwT = np.ascontiguousarray((FS * W0 * np.asarray(Ww, np.float32)).T).astype(f8)
    wwT = wwT.reshape(2, 2, 128, H).transpose(2, 0, 1, 3)
    wuT = np.ascontiguousarray((FS * W0 * np.asarray(Wu, np.float32)).T).astype(f8)
    wuT = wuT.reshape(2, 2, 128, H).transpose(2, 0, 1, 3)

    wv = np.asarray(Wv, np.float32)[0]
    wvc = np.zeros((128, NHC, 4), np.float32)
    for hc in range(NHC):
        for k in range(4):
            wvc[:, hc, k] = COEF[k] * wv[128 * hc:128 * hc + 128]

    jj = np.arange(BAND)[None, :]
    ll = np.arange(128)[:, None]
    dd = jj - ll
    mask = (((dd >= 0) & (dd <= 2 * P)) & (dd != P)).astype(bf)

    eye = np.eye(128, dtype=bf)

    in_maps = []
    for m in range(M):
        xh = x_pad[LLOC * m: LLOC * m + HALO].astype(bf)
        xh_a = np.zeros((128, NLC + 1, D), bf)
        xh_a[:, :NLC] = xh[:512].reshape(NLC, 128, D).transpose(1, 0, 2)
        xh_a[0:32, NLC] = xh[512:HALO]
        xT = np.ascontiguousarray(x_pad[LLOC * m: LLOC * m + HALO].T).astype(f8)
        xT_a = xT.reshape(2, 2, 128, HALO).transpose(2, 0, 1, 3)
        in_maps.append({
            "xT": np.ascontiguousarray(xT_a),
            "xh": np.ascontiguousarray(xh_a),
            "wwT": np.ascontiguousarray(wwT),
            "wuT": np.ascontiguousarray(wuT),
            "wvc": wvc,
            "mask": np.ascontiguousarray(mask),
            "eye": eye,
        })
    return in_maps


def assemble_out(results):
    shards = []
    for m in range(M):
        o = np.asarray(results[m]["out"]).reshape(128, NLC, D)
        shards.append(o.transpose(1, 0, 2).reshape(LLOC, D))
    return np.concatenate(shards, 0).astype(np.float32)


def kernel(x, Ww, Wu, Wv):
    nc = build_nc()
    in_maps = make_in_maps(x, Ww, Wu, Wv)
    res = bass_utils.run_bass_kernel_spmd(nc, in_maps, core_ids=list(range(M)))
    return assemble_out(res.results)
